# revision 29
# baseline (speedup 1.0000x reference)
"""Multi-head attention forward (B=4, S=2048, D=768, H=12, hd=64) on 8 trn2 cores.

Sharding: core c handles batch b=c//2 and head-group hg=c%2 (6 heads each).
Each core computes its 6 heads' attention and a *partial* output projection
(over its 384 columns of w_proj). Host sums the two head-group partials per
batch and adds b_proj.

Per-core dataflow (all matmuls in float32r, full PE rate at N>=256):
  xT (6x[128,2048])  --wqkT-->  QK^T packed per head [128(q64|k64), 2048]
  xT                 --wvT--->  V natural [128tok, 16kb, 6h, 65] (col 64 = ones)
  S^T[k,q] = matmul(lhsT=K^T chunk, rhs=Q^T)          (PSUM [128,512])
  P^T = exp(S^T * 0.125)                              (ACT -> SBUF)
  O^T[65,q] += matmul(lhsT=V_aug[kb], rhs=P^T)        (row 64 = softmax sums)
  recip-broadcast via ones-matmul, normalize -> OT_sb [384, 2048]
  outT partial [768,2048] = matmul(lhsT=wpsT chunks, rhs=OT chunks) -> DRAM
"""

import sys

import numpy as np

if "/opt/trn_rl_repo" not in sys.path:
    sys.path.insert(0, "/opt/trn_rl_repo")

import concourse.bacc as bacc
import concourse.bass as bass
import concourse.mybir as mybir
import concourse.tile as tile
from concourse.bass_utils import run_bass_kernel_spmd

F32 = mybir.dt.float32
F32R = mybir.dt.float32r

S = 2048
D = 768
HL = 6          # heads per core
HD = 64
NQC = 4         # q chunks of 512
NKB = 16        # k blocks of 128
SCALE = 0.125   # 64 ** -0.5

_CACHE = {}


OT_BUFS = 2
AUX_BUFS = 2


def build_nc(unroll=1):
    nc = bacc.Bacc(None, target_bir_lowering=False)
    xt_d = nc.dram_tensor("xt", [6, 128, S], F32R, kind="ExternalInput")
    wqk_d = nc.dram_tensor("wqk", [6, 128, D], F32R, kind="ExternalInput")
    bqk_d = nc.dram_tensor("bqk", [128, HL], F32, kind="ExternalInput")
    wv_d = nc.dram_tensor("wv", [6, 128, HL * (HD + 1)], F32R, kind="ExternalInput")
    bv_d = nc.dram_tensor("bv", [1, HL * (HD + 1)], F32R, kind="ExternalInput")
    ones_d = nc.dram_tensor("ones", [1, 128], F32R, kind="ExternalInput")
    wps_d = nc.dram_tensor("wps", [3, 128, D], F32R, kind="ExternalInput")
    out_d = nc.dram_tensor("outT", [D, S], F32, kind="ExternalOutput")

    with tile.TileContext(nc) as tc:
        with (
            tc.tile_pool(name="const", bufs=1) as constp,
            tc.tile_pool(name="big", bufs=1) as bigp,
            tc.tile_pool(name="pt", bufs=3) as ptp,
            tc.tile_pool(name="small", bufs=2) as smallp,
            tc.tile_pool(name="osbp", bufs=2) as osbp,
            tc.tile_pool(name="evp", bufs=3) as evp,
            tc.tile_pool(name="mm", bufs=2, space="PSUM") as mmp,
            tc.tile_pool(name="otp", bufs=OT_BUFS, space="PSUM") as otp,
            tc.tile_pool(name="aux", bufs=AUX_BUFS, space="PSUM") as auxp,
        ):
            emit_all(nc, tc, constp, bigp, ptp, smallp, osbp, mmp, otp, auxp,
                     xt_d, wqk_d, bqk_d, wv_d, bv_d, ones_d, wps_d, out_d,
                     unroll, evp)
    nc.compile()
    return nc


def emit_all(nc, tc, constp, bigp, ptp, smallp, osbp, mmp, otp, auxp,
             xt_d, wqk_d, bqk_d, wv_d, bv_d, ones_d, wps_d, out_d, unroll, evp):
    for _it in range(unroll):
            # ---- weights first (small), then x^T in half-row DMAs ----
            wqks = []
            for c in range(6):
                t = constp.tile([128, D], F32R, tag=f"wqk{c}")
                nc.sync.dma_start(out=t, in_=wqk_d[c])
                wqks.append(t)
            bqk_sb = constp.tile([128, HL], F32, tag="bqk")
            nc.sync.dma_start(out=bqk_sb, in_=bqk_d[:, :])
            bv_sb = constp.tile([1, HL * (HD + 1)], F32R, tag="bv")
            nc.sync.dma_start(out=bv_sb, in_=bv_d[:, :])
            ones_sb = constp.tile([1, 128], F32R, tag="ones")
            nc.sync.dma_start(out=ones_sb, in_=ones_d[:, :])
            xts = [[None] * NQC for _ in range(6)]  # [c][q]: [128,512] x^T block
            for q in range(NQC):
                for c in range(6):
                    t = bigp.tile(
                        [128, 512], F32R, tag=f"xt{c}_{q}", name=f"xt{c}_{q}"
                    )
                    nc.sync.dma_start(
                        out=t, in_=xt_d[c, :, q * 512 : (q + 1) * 512]
                    )
                    xts[c][q] = t
                if q == 0:
                    wvs = []
                    for c in range(6):
                        t = constp.tile([128, HL * (HD + 1)], F32R, tag=f"wv{c}")
                        nc.sync.dma_start(out=t, in_=wv_d[c])
                        wvs.append(t)

            # per-kb V tiles so PV(kb) only waits on its own generation group
            v_sb = [
                bigp.tile([128, HL, HD + 1], F32R, tag=f"vsb{t}", name=f"vsb{t}")
                for t in range(NKB)
            ]
            # per pair: K^T [128, S] and per-qc Q^T [128, 512]
            # (partitions g*64:(g+1)*64 = head 2i+g)
            qk_k = [
                [
                    bigp.tile([128, 512], F32R, tag=f"qkk{i}_{q}", name=f"qkk{i}_{q}")
                    for q in range(NQC)
                ]
                for i in range(HL // 2)
            ]
            qk_q = [
                [
                    bigp.tile([128, 512], F32R, tag=f"qkq{i}_{q}", name=f"qkq{i}_{q}")
                    for q in range(NQC)
                ]
                for i in range(HL // 2)
            ]
            # OT chunks per qc: ots[0] own slots; ots[1]/ots[2] reuse qkk[0]/qkk[1]
            ots = [
                [
                    bigp.tile([128, 512], F32R, tag=f"ots0_{q}", name=f"ots0_{q}")
                    for q in range(NQC)
                ]
            ]

            def gen_qk_group(i, g, q, pool=None):
                pool = pool if pool is not None else auxp
                ps = pool.tile(
                    [128, 512], F32,
                    tag="aux" if pool is auxp else "mm",
                    name="ps",
                )
                for c in range(6):
                    nc.tensor.matmul(
                        ps,
                        lhsT=wqks[c][
                            :, i * 256 + g * 128 : i * 256 + (g + 1) * 128
                        ],
                        rhs=xts[c][q],
                        start=(c == 0),
                        stop=(c == 5),
                    )
                dst = qk_q[i][q] if g == 0 else qk_k[i][q]
                nc.vector.tensor_scalar_add(
                    out=dst, in0=ps, scalar1=bqk_sb[:, 2 * i + g : 2 * i + g + 1]
                )

            def gen_v_group(t):
                ps = auxp.tile([128, HL * (HD + 1)], F32, tag="aux", name="ps")
                tq, tr = divmod(t, 4)
                for c in range(6):
                    nc.tensor.matmul(
                        ps,
                        lhsT=xts[c][tq][:, tr * 128 : (tr + 1) * 128],
                        rhs=wvs[c],
                        start=(c == 0),
                        stop=False,
                    )
                nc.tensor.matmul(ps, lhsT=ones_sb, rhs=bv_sb, start=False, stop=True)
                nc.vector.tensor_copy(
                    out=v_sb[t], in_=ps.rearrange("p (h d) -> p h d", h=HL)
                )

            wpss = []

            def proj_group(q, m):
                qs = slice(q * 512, (q + 1) * 512)
                pp = auxp.tile([128, 512], F32, tag="aux", name="pp")
                for c in range(3):
                    nc.tensor.matmul(
                        pp,
                        lhsT=wpss[c][:, m * 128 : (m + 1) * 128],
                        rhs=ots[c][q],
                        start=(c == 0),
                        stop=(c == 2),
                    )
                osb = osbp.tile([128, 512], F32, tag="outsb", name="osb")
                nc.vector.tensor_copy(out=osb, in_=pp)
                nc.sync.dma_start(out=out_d[m * 128 : (m + 1) * 128, qs], in_=osb)

            def attn(i, q, filler=None):
                ot_ab = [
                    otp.tile([HD + 1, 512], F32, tag="ot", name=f"ot{g}")
                    for g in range(2)
                ]
                for kb in range(NKB):
                    # the two heads of the pair run concurrently on PE:
                    # row-tiles (0,0) and (64,0), separate PSUM banks
                    st2 = mmp.tile([128, 2, 512], F32, tag="mm", name="st2")
                    pt2 = ptp.tile([128, 2, 512], F32R, tag="pt", name="pt2")
                    for g in range(2):
                        prange = slice(g * 64, (g + 1) * 64)
                        kq, kr = divmod(kb, 4)
                        nc.tensor.matmul(
                            st2[:, g, :],
                            lhsT=qk_k[i][kq][prange, kr * 128 : (kr + 1) * 128],
                            rhs=qk_q[i][q][prange, :],
                            start=True,
                            stop=True,
                        )
                    nc.scalar.activation(
                        out=pt2,
                        in_=st2,
                        func=mybir.ActivationFunctionType.Exp,
                        scale=SCALE,
                    )
                    for g in range(2):
                        nc.tensor.matmul(
                            ot_ab[g],
                            lhsT=v_sb[kb][:, 2 * i + g, :],
                            rhs=pt2[:, g, :],
                            start=(kb == 0),
                            stop=(kb == NKB - 1),
                        )
                    if filler is not None:
                        filler(kb)
                # evict O^T to SBUF right away (frees the PSUM slot), then
                # normalize: row HD holds sum_k exp; reciprocal on the single
                # row, broadcast via GPSIMD, scale rows
                for g in range(2):
                    ot_sb = evp.tile([HD + 1, 512], F32, tag="otsb", name="ot_sb")
                    nc.vector.tensor_copy(out=ot_sb, in_=ot_ab[g])
                    rcp = smallp.tile([1, 512], F32, tag="sums")
                    nc.vector.reciprocal(out=rcp, in_=ot_sb[HD : HD + 1, :])
                    rec = smallp.tile([64, 512], F32, tag="rec")
                    nc.gpsimd.partition_broadcast(out_ap=rec, in_ap=rcp)
                    nc.vector.tensor_mul(
                        out=ots[i][q][g * 64 : (g + 1) * 64, :],
                        in0=ot_sb[0:HD, :],
                        in1=rec,
                    )

            # pair 0: only K(q0) + Q(q0) + V(0,1) up front; K(q1..3),
            # Q(q1..3) and V(2..15) are interleaved into attn(0,0)'s kb
            # stream, each placed just before the kb that first needs it
            # (and after its xt DMA lands).
            gen_qk_group(0, 1, 0, pool=auxp)
            gen_qk_group(0, 0, 0, pool=mmp)
            gen_v_group(0)
            gen_v_group(1)

            def filler_v(kb):
                if kb in (1, 5, 9):  # K(0, q1..3) before kb 4/8/12 need them
                    gen_qk_group(0, 1, 1 + (kb - 1) // 4, pool=mmp)
                if kb + 2 < NKB:
                    gen_v_group(kb + 2)
                if kb % 5 == 4:  # kb 4, 9, 14 -> Q(0, q1..3)
                    gen_qk_group(0, 0, 1 + kb // 5, pool=mmp)

            def make_filler_qk(i_next, half):
                def f(kb):
                    if kb % 4 == 0:
                        g, q4 = divmod(half * 4 + kb // 4, 4)
                        gen_qk_group(i_next, g, q4)
                return f

            def make_filler_proj(q_src):
                def f(kb):
                    if kb % 3 == 0 and kb // 3 < 6:
                        proj_group(q_src, kb // 3)
                return f

            for i in range(HL // 2):
                if i > 0:
                    # ots[i] reuses qkk[i-1]'s slots (released after attn(i-1))
                    ots.append(
                        [
                            bigp.tile(
                                [128, 512], F32R,
                                tag=f"qkk{i-1}_{q}", name=f"ots{i}_{q}",
                            )
                            for q in range(NQC)
                        ]
                    )
                if i == 2 and not wpss:
                    for c in range(3):
                        t = constp.tile(
                            [128, D], F32R, tag=f"wqk{c}", name=f"wps{c}"
                        )
                        nc.sync.dma_start(out=t, in_=wps_d[c])
                        wpss.append(t)
                for q in range(NQC):
                    if i == 0 and q == 0:
                        filler = filler_v
                    elif i < 2 and q in (1, 2):
                        filler = make_filler_qk(i + 1, q - 1)
                    elif i == 2 and q > 0:
                        filler = make_filler_proj(q - 1)
                    else:
                        filler = None
                    attn(i, q, filler)
            for m in range(6):
                proj_group(NQC - 1, m)


def make_in_maps(x, w_qkv, b_qkv, w_proj):
    x = np.asarray(x, np.float32)
    w_qkv = np.asarray(w_qkv, np.float32)
    b_qkv = np.asarray(b_qkv, np.float32)
    w_proj = np.asarray(w_proj, np.float32)
    in_maps = []
    for core in range(8):
        b, hg = divmod(core, 2)
        h0 = HL * hg
        heads = list(range(h0, h0 + HL))
        xt = np.ascontiguousarray(x[b].T).reshape(6, 128, S)
        # free layout per pair i: [Q_2i | Q_2i+1 | K_2i | K_2i+1] (128+128)
        A = np.empty((D, D), np.float32)
        bqk = np.empty((128, HL), np.float32)
        for i in range(HL // 2):
            ha, hb = heads[2 * i], heads[2 * i + 1]
            for g, off in enumerate((0, D)):  # 0 = Q rows, D = K rows
                base = i * 256 + g * 128
                A[:, base : base + 64] = w_qkv[off + 64 * ha : off + 64 * ha + 64].T
                A[:, base + 64 : base + 128] = w_qkv[off + 64 * hb : off + 64 * hb + 64].T
                bqk[0:64, 2 * i + g] = b_qkv[off + 64 * ha : off + 64 * ha + 64]
                bqk[64:128, 2 * i + g] = b_qkv[off + 64 * hb : off + 64 * hb + 64]
        wqk = np.ascontiguousarray(A.reshape(6, 128, D))
        Av = np.zeros((D, HL * (HD + 1)), np.float32)
        bv = np.zeros((1, HL * (HD + 1)), np.float32)
        for j, h in enumerate(heads):
            Av[:, j * 65 : j * 65 + 64] = w_qkv[2 * D + 64 * h : 2 * D + 64 * h + 64].T
            bv[0, j * 65 : j * 65 + 64] = b_qkv[2 * D + 64 * h : 2 * D + 64 * h + 64]
            bv[0, j * 65 + 64] = 1.0
        wv = np.ascontiguousarray(Av.reshape(D // 128, 128, HL * (HD + 1)))
        wps = np.ascontiguousarray(
            w_proj[:, 64 * h0 : 64 * h0 + HL * HD].T.reshape(3, 128, D)
        )
        in_maps.append(
            {
                "xt": xt,
                "wqk": wqk,
                "bqk": bqk,
                "wv": wv,
                "bv": bv,
                "wps": wps,
                "ones": np.ones((1, 128), np.float32),
            }
        )
    return in_maps


def kernel(x, w_qkv, b_qkv, w_proj, b_proj, _trace=False, _tmpdir=None):
    if "nc" not in _CACHE:
        _CACHE["nc"] = build_nc()
    nc = _CACHE["nc"]
    in_maps = make_in_maps(x, w_qkv, b_qkv, w_proj)
    res = run_bass_kernel_spmd(
        nc,
        in_maps,
        core_ids=list(range(8)),
        trace=_trace,
        tmpdir=_tmpdir,
        **({"trace_cores": [0], "stitch_traces": False} if _trace else {}),
    )
    _CACHE["last_result"] = res
    b_proj = np.asarray(b_proj, np.float32)
    out = np.empty((4, S, D), np.float32)
    for b in range(4):
        out[b] = (res.results[2 * b]["outT"] + res.results[2 * b + 1]["outT"]).T
        out[b] += b_proj
    return out


# revision 30
# speedup vs baseline: 1.2854x; 1.2854x over previous
"""Multi-head attention forward (B=4, S=2048, D=768, H=12, hd=64) on 8 trn2 cores.

Sharding: core c handles batch b=c//2 and head-group hg=c%2 (6 heads each).
Each core computes its 6 heads' attention and a *partial* output projection
(over its 384 columns of w_proj). Host sums the two head-group partials per
batch and adds b_proj.

Per-core dataflow (all matmuls in float32r, full PE rate at N>=256):
  xT (6x[128,2048])  --wqkT-->  QK^T packed per head [128(q64|k64), 2048]
  xT                 --wvT--->  V natural [128tok, 16kb, 6h, 65] (col 64 = ones)
  S^T[k,q] = matmul(lhsT=K^T chunk, rhs=Q^T)          (PSUM [128,512])
  P^T = exp(S^T * 0.125)                              (ACT -> SBUF)
  O^T[65,q] += matmul(lhsT=V_aug[kb], rhs=P^T)        (row 64 = softmax sums)
  recip-broadcast via ones-matmul, normalize -> OT_sb [384, 2048]
  outT partial [768,2048] = matmul(lhsT=wpsT chunks, rhs=OT chunks) -> DRAM
"""

import sys

import numpy as np

if "/opt/trn_rl_repo" not in sys.path:
    sys.path.insert(0, "/opt/trn_rl_repo")

import concourse.bacc as bacc
import concourse.bass as bass
import concourse.mybir as mybir
import concourse.tile as tile
from concourse.bass_utils import run_bass_kernel_spmd

F32 = mybir.dt.float32
F32R = mybir.dt.float32r

S = 2048
D = 768
HL = 6          # heads per core
HD = 64
NQC = 4         # q chunks of 512
NKB = 16        # k blocks of 128
SCALE = 0.125   # 64 ** -0.5

_CACHE = {}


OT_BUFS = 2
AUX_BUFS = 2


def build_nc(unroll=1):
    nc = bacc.Bacc(None, target_bir_lowering=False)
    xt_d = nc.dram_tensor("xt", [6, 128, S], F32R, kind="ExternalInput")
    wqk_d = nc.dram_tensor("wqk", [6, 128, D], F32R, kind="ExternalInput")
    bqk_d = nc.dram_tensor("bqk", [128, HL], F32, kind="ExternalInput")
    wv_d = nc.dram_tensor("wv", [6, 128, HL * (HD + 1)], F32R, kind="ExternalInput")
    bv_d = nc.dram_tensor("bv", [1, HL * (HD + 1)], F32R, kind="ExternalInput")
    ones_d = nc.dram_tensor("ones", [1, 128], F32R, kind="ExternalInput")
    wps_d = nc.dram_tensor("wps", [3, 128, D], F32R, kind="ExternalInput")
    out_d = nc.dram_tensor("outT", [D, S], F32, kind="ExternalOutput")

    with tile.TileContext(nc) as tc:
        with (
            tc.tile_pool(name="const", bufs=1) as constp,
            tc.tile_pool(name="big", bufs=1) as bigp,
            tc.tile_pool(name="pt", bufs=3) as ptp,
            tc.tile_pool(name="small", bufs=2) as smallp,
            tc.tile_pool(name="osbp", bufs=2) as osbp,
            tc.tile_pool(name="evp", bufs=3) as evp,
            tc.tile_pool(name="mm", bufs=2, space="PSUM") as mmp,
            tc.tile_pool(name="otp", bufs=OT_BUFS, space="PSUM") as otp,
            tc.tile_pool(name="aux", bufs=AUX_BUFS, space="PSUM") as auxp,
        ):
            emit_all(nc, tc, constp, bigp, ptp, smallp, osbp, mmp, otp, auxp,
                     xt_d, wqk_d, bqk_d, wv_d, bv_d, ones_d, wps_d, out_d,
                     unroll, evp)
    nc.compile()
    return nc


def emit_all(nc, tc, constp, bigp, ptp, smallp, osbp, mmp, otp, auxp,
             xt_d, wqk_d, bqk_d, wv_d, bv_d, ones_d, wps_d, out_d, unroll, evp):
    for _it in range(unroll):
            # ---- weights first (small), then x^T in half-row DMAs ----
            wqks = []
            for c in range(6):
                t = constp.tile([128, D], F32R, tag=f"wqk{c}")
                nc.sync.dma_start(out=t, in_=wqk_d[c])
                wqks.append(t)
            bqk_sb = constp.tile([128, HL], F32, tag="bqk")
            nc.sync.dma_start(out=bqk_sb, in_=bqk_d[:, :])
            bv_sb = constp.tile([1, HL * (HD + 1)], F32R, tag="bv")
            nc.sync.dma_start(out=bv_sb, in_=bv_d[:, :])
            ones_sb = constp.tile([1, 128], F32R, tag="ones")
            nc.sync.dma_start(out=ones_sb, in_=ones_d[:, :])
            xts = [[None] * NQC for _ in range(6)]  # [c][q]: [128,512] x^T block
            for q in range(NQC):
                for c in range(6):
                    t = bigp.tile(
                        [128, 512], F32R, tag=f"xt{c}_{q}", name=f"xt{c}_{q}"
                    )
                    nc.sync.dma_start(
                        out=t, in_=xt_d[c, :, q * 512 : (q + 1) * 512]
                    )
                    xts[c][q] = t
                if q == 0:
                    wvs = []
                    for c in range(6):
                        t = constp.tile([128, HL * (HD + 1)], F32R, tag=f"wv{c}")
                        nc.sync.dma_start(out=t, in_=wv_d[c])
                        wvs.append(t)

            # per-kb V tiles so PV(kb) only waits on its own generation group
            v_sb = [
                bigp.tile([128, HL, HD + 1], F32R, tag=f"vsb{t}", name=f"vsb{t}")
                for t in range(NKB)
            ]
            # per pair: K^T [128, S] and per-qc Q^T [128, 512]
            # (partitions g*64:(g+1)*64 = head 2i+g)
            qk_k = [
                [
                    bigp.tile([128, 512], F32R, tag=f"qkk{i}_{q}", name=f"qkk{i}_{q}")
                    for q in range(NQC)
                ]
                for i in range(HL // 2)
            ]
            qk_q = [
                [
                    bigp.tile([128, 512], F32R, tag=f"qkq{i}_{q}", name=f"qkq{i}_{q}")
                    for q in range(NQC)
                ]
                for i in range(HL // 2)
            ]
            # OT chunks per qc: ots[0] own slots; ots[1]/ots[2] reuse qkk[0]/qkk[1]
            ots = [
                [
                    bigp.tile([128, 512], F32R, tag=f"ots0_{q}", name=f"ots0_{q}")
                    for q in range(NQC)
                ]
            ]

            def gen_qk_group(i, g, q, pool=None):
                pool = pool if pool is not None else auxp
                ps = pool.tile(
                    [128, 512], F32,
                    tag="aux" if pool is auxp else "mm",
                    name="ps",
                )
                for c in range(6):
                    nc.tensor.matmul(
                        ps,
                        lhsT=wqks[c][
                            :, i * 256 + g * 128 : i * 256 + (g + 1) * 128
                        ],
                        rhs=xts[c][q],
                        start=(c == 0),
                        stop=(c == 5),
                    )
                dst = qk_q[i][q] if g == 0 else qk_k[i][q]
                nc.vector.tensor_scalar_add(
                    out=dst, in0=ps, scalar1=bqk_sb[:, 2 * i + g : 2 * i + g + 1]
                )

            def gen_v_group(t):
                ps = auxp.tile([128, HL * (HD + 1)], F32, tag="aux", name="ps")
                tq, tr = divmod(t, 4)
                for c in range(6):
                    nc.tensor.matmul(
                        ps,
                        lhsT=xts[c][tq][:, tr * 128 : (tr + 1) * 128],
                        rhs=wvs[c],
                        start=(c == 0),
                        stop=False,
                    )
                nc.tensor.matmul(ps, lhsT=ones_sb, rhs=bv_sb, start=False, stop=True)
                nc.vector.tensor_copy(
                    out=v_sb[t], in_=ps.rearrange("p (h d) -> p h d", h=HL)
                )

            wpss = []

            def proj_group(q, m):
                qs = slice(q * 512, (q + 1) * 512)
                pp = auxp.tile([128, 512], F32, tag="aux", name="pp")
                for c in range(3):
                    nc.tensor.matmul(
                        pp,
                        lhsT=wpss[c][:, m * 128 : (m + 1) * 128],
                        rhs=ots[c][q],
                        start=(c == 0),
                        stop=(c == 2),
                    )
                osb = osbp.tile([128, 512], F32, tag="outsb", name="osb")
                nc.vector.tensor_copy(out=osb, in_=pp)
                nc.sync.dma_start(out=out_d[m * 128 : (m + 1) * 128, qs], in_=osb)

            def attn(i, q, filler=None):
                ot_ab = [
                    otp.tile([HD + 1, 512], F32, tag="ot", name=f"ot{g}")
                    for g in range(2)
                ]
                prev = None  # PV runs one kb behind so PE never HOL-blocks
                for kb in range(NKB):
                    # the two heads of the pair run concurrently on PE:
                    # row-tiles (0,0) and (64,0), separate PSUM banks
                    st2 = mmp.tile([128, 2, 512], F32, tag="mm", name="st2")
                    pt2 = ptp.tile([128, 2, 512], F32R, tag="pt", name="pt2")
                    for g in range(2):
                        prange = slice(g * 64, (g + 1) * 64)
                        kq, kr = divmod(kb, 4)
                        nc.tensor.matmul(
                            st2[:, g, :],
                            lhsT=qk_k[i][kq][prange, kr * 128 : (kr + 1) * 128],
                            rhs=qk_q[i][q][prange, :],
                            start=True,
                            stop=True,
                        )
                    nc.scalar.activation(
                        out=pt2,
                        in_=st2,
                        func=mybir.ActivationFunctionType.Exp,
                        scale=SCALE,
                    )
                    if prev is not None:
                        pkb, ppt = prev
                        for g in range(2):
                            nc.tensor.matmul(
                                ot_ab[g],
                                lhsT=v_sb[pkb][:, 2 * i + g, :],
                                rhs=ppt[:, g, :],
                                start=(pkb == 0),
                                stop=False,
                            )
                    prev = (kb, pt2)
                    if filler is not None:
                        filler(kb)
                pkb, ppt = prev
                for g in range(2):
                    nc.tensor.matmul(
                        ot_ab[g],
                        lhsT=v_sb[pkb][:, 2 * i + g, :],
                        rhs=ppt[:, g, :],
                        start=False,
                        stop=True,
                    )
                # evict O^T to SBUF right away (frees the PSUM slot), then
                # normalize: row HD holds sum_k exp; reciprocal on the single
                # row, broadcast via GPSIMD, scale rows
                for g in range(2):
                    ot_sb = evp.tile([HD + 1, 512], F32, tag="otsb", name="ot_sb")
                    nc.vector.tensor_copy(out=ot_sb, in_=ot_ab[g])
                    rcp = smallp.tile([1, 512], F32, tag="sums")
                    nc.vector.reciprocal(out=rcp, in_=ot_sb[HD : HD + 1, :])
                    rec = smallp.tile([64, 512], F32, tag="rec")
                    nc.gpsimd.partition_broadcast(out_ap=rec, in_ap=rcp)
                    nc.vector.tensor_mul(
                        out=ots[i][q][g * 64 : (g + 1) * 64, :],
                        in0=ot_sb[0:HD, :],
                        in1=rec,
                    )

            # pair 0: only K(q0) + Q(q0) + V(0,1) up front; K(q1..3),
            # Q(q1..3) and V(2..15) are interleaved into attn(0,0)'s kb
            # stream, each placed just before the kb that first needs it
            # (and after its xt DMA lands).
            gen_qk_group(0, 1, 0, pool=auxp)
            gen_qk_group(0, 0, 0, pool=mmp)
            gen_v_group(0)
            gen_v_group(1)

            def filler_v(kb):
                if kb in (1, 5, 9):  # K(0, q1..3) before kb 4/8/12 need them
                    gen_qk_group(0, 1, 1 + (kb - 1) // 4, pool=mmp)
                if kb + 2 < NKB:
                    gen_v_group(kb + 2)
                if kb % 5 == 4:  # kb 4, 9, 14 -> Q(0, q1..3)
                    gen_qk_group(0, 0, 1 + kb // 5, pool=mmp)

            def make_filler_qk(i_next, half):
                def f(kb):
                    if kb % 4 == 0:
                        g, q4 = divmod(half * 4 + kb // 4, 4)
                        gen_qk_group(i_next, g, q4)
                return f

            def make_filler_proj(q_src):
                def f(kb):
                    if kb % 3 == 0 and kb // 3 < 6:
                        proj_group(q_src, kb // 3)
                return f

            for i in range(HL // 2):
                if i > 0:
                    # ots[i] reuses qkk[i-1]'s slots (released after attn(i-1))
                    ots.append(
                        [
                            bigp.tile(
                                [128, 512], F32R,
                                tag=f"qkk{i-1}_{q}", name=f"ots{i}_{q}",
                            )
                            for q in range(NQC)
                        ]
                    )
                if i == 2 and not wpss:
                    for c in range(3):
                        t = constp.tile(
                            [128, D], F32R, tag=f"wqk{c}", name=f"wps{c}"
                        )
                        nc.sync.dma_start(out=t, in_=wps_d[c])
                        wpss.append(t)
                for q in range(NQC):
                    if i == 0 and q == 0:
                        filler = filler_v
                    elif i < 2 and q in (1, 2):
                        filler = make_filler_qk(i + 1, q - 1)
                    elif i == 2 and q > 0:
                        filler = make_filler_proj(q - 1)
                    else:
                        filler = None
                    attn(i, q, filler)
            for m in range(6):
                proj_group(NQC - 1, m)


def make_in_maps(x, w_qkv, b_qkv, w_proj):
    x = np.asarray(x, np.float32)
    w_qkv = np.asarray(w_qkv, np.float32)
    b_qkv = np.asarray(b_qkv, np.float32)
    w_proj = np.asarray(w_proj, np.float32)
    in_maps = []
    for core in range(8):
        b, hg = divmod(core, 2)
        h0 = HL * hg
        heads = list(range(h0, h0 + HL))
        xt = np.ascontiguousarray(x[b].T).reshape(6, 128, S)
        # free layout per pair i: [Q_2i | Q_2i+1 | K_2i | K_2i+1] (128+128)
        A = np.empty((D, D), np.float32)
        bqk = np.empty((128, HL), np.float32)
        for i in range(HL // 2):
            ha, hb = heads[2 * i], heads[2 * i + 1]
            for g, off in enumerate((0, D)):  # 0 = Q rows, D = K rows
                base = i * 256 + g * 128
                A[:, base : base + 64] = w_qkv[off + 64 * ha : off + 64 * ha + 64].T
                A[:, base + 64 : base + 128] = w_qkv[off + 64 * hb : off + 64 * hb + 64].T
                bqk[0:64, 2 * i + g] = b_qkv[off + 64 * ha : off + 64 * ha + 64]
                bqk[64:128, 2 * i + g] = b_qkv[off + 64 * hb : off + 64 * hb + 64]
        wqk = np.ascontiguousarray(A.reshape(6, 128, D))
        Av = np.zeros((D, HL * (HD + 1)), np.float32)
        bv = np.zeros((1, HL * (HD + 1)), np.float32)
        for j, h in enumerate(heads):
            Av[:, j * 65 : j * 65 + 64] = w_qkv[2 * D + 64 * h : 2 * D + 64 * h + 64].T
            bv[0, j * 65 : j * 65 + 64] = b_qkv[2 * D + 64 * h : 2 * D + 64 * h + 64]
            bv[0, j * 65 + 64] = 1.0
        wv = np.ascontiguousarray(Av.reshape(D // 128, 128, HL * (HD + 1)))
        wps = np.ascontiguousarray(
            w_proj[:, 64 * h0 : 64 * h0 + HL * HD].T.reshape(3, 128, D)
        )
        in_maps.append(
            {
                "xt": xt,
                "wqk": wqk,
                "bqk": bqk,
                "wv": wv,
                "bv": bv,
                "wps": wps,
                "ones": np.ones((1, 128), np.float32),
            }
        )
    return in_maps


def kernel(x, w_qkv, b_qkv, w_proj, b_proj, _trace=False, _tmpdir=None):
    if "nc" not in _CACHE:
        _CACHE["nc"] = build_nc()
    nc = _CACHE["nc"]
    in_maps = make_in_maps(x, w_qkv, b_qkv, w_proj)
    res = run_bass_kernel_spmd(
        nc,
        in_maps,
        core_ids=list(range(8)),
        trace=_trace,
        tmpdir=_tmpdir,
        **({"trace_cores": [0], "stitch_traces": False} if _trace else {}),
    )
    _CACHE["last_result"] = res
    b_proj = np.asarray(b_proj, np.float32)
    out = np.empty((4, S, D), np.float32)
    for b in range(4):
        out[b] = (res.results[2 * b]["outT"] + res.results[2 * b + 1]["outT"]).T
        out[b] += b_proj
    return out
